# revision 16
# baseline (speedup 1.0000x reference)
"""Trainium2 Bass kernel for nn_BusDecoder (moe_routing).

Computes out[b, n*2+o] = sum_d H[b,n,d] * W[t_n, d, o] + b[t_n, o] with
t_n = bus_type[0, n], for B=32, N=4096, D=1024, OUT=2, 3 types.

Strategy (memory-bound regime):
  - Data-parallel over batch B across 8 cores (B_local=4 per core).
  - H streams as f16 (absmax-rel err ~2e-4 vs the 2e-2 gate); W rides as a
    12-column f16 stack [Whi | Wlo] with Wlo the f32-f16 residual, so the
    einsum sees W at effectively full f32 precision for free (the mask stage
    sums both halves).
  - H is pre-tiled on the host so every DMA block is one contiguous run per
    partition (this is the main memory-rate lever).
  - Per 512-token group: 8 accumulating f16 matmuls [K=128, M=12, N=512]
    into PSUM, then one VectorE scalar_tensor_tensor applies bias + one-hot
    routing mask (by bus type) writing f16 directly, and a single f16 matmul
    with a constant 0/1 matrix T[12, 2] pair-sums into out[2, 512] in PSUM;
    the Activation engine copies PSUM->SBUF and stores are batched 8 groups
    per DMA on the scalar HWDGE ring so they rarely contend with H loads on
    the shared SDMA engines. The select for group g is emitted one group
    late so the PE never waits on VectorE.
  - Deep buffering (12 H-block buffers, 4 PSUM groups) keeps the sync-ring
    DMA queue fed end-to-end; measured per-exec time tracks the pure-DMA
    rate of the H stream (~385-437 GB/s/core depending on machine load).
"""

import numpy as np

import concourse.bacc as bacc
import concourse.bass_utils as bass_utils
import concourse.mybir as mybir
import concourse.tile as tile

B, N, D, OUT = 32, 4096, 1024, 2
N_TYPES = 3
N_CORES = 8
BL = B // N_CORES          # 4 batch rows per core
TOK = BL * N               # 16384 tokens per core
P = 128
DCH = D // P               # 8 contraction chunks
G = 512                    # tokens per matmul group (one PSUM bank of fp32)
NG = TOK // G              # 32 groups
C12 = 12                   # weight stack width (Whi | Wlo)

BT = 512                   # tokens per H DMA block (host pretile granularity)

_CACHED_NC = {}


def _build_nc(repeat=1, bt=BT, hbufs=12, split_dma=False, psbufs=4,
              dual_ring=False, mode="full", g=G, wkbufs=3, ostore=8):
    # repeat>1 wraps the body in a device-side For_i loop running the
    # identical workload `repeat` times — used only by test.py to measure
    # per-execution hardware time through the high-latency axon tunnel.
    # mode: "full" | "dma" (loads only) | "compute" (loads once, loops math)
    key = (repeat, bt, hbufs, split_dma, psbufs, dual_ring, mode, g, wkbufs, ostore)
    if key in _CACHED_NC:
        return _CACHED_NC[key]

    f16 = mybir.dt.float16
    f32 = mybir.dt.float32

    nc = bacc.Bacc("TRN2", debug=False)
    # h2 is host-pre-tiled to the exact per-block SBUF layout so each block
    # DMA reads one contiguous run per partition:
    #   h2[c, p, do, t] = (H^T)[do*128+p, c*BT+t]
    h2 = nc.dram_tensor("h2", [TOK // bt, P, DCH, bt], f16,
                        kind="ExternalInput")
    wstk = nc.dram_tensor("wstk", [D, C12], f16, kind="ExternalInput")
    bvec = nc.dram_tensor("bvec", [C12, 1], f32, kind="ExternalInput")
    mask = nc.dram_tensor("mask12", [C12, TOK], f16, kind="ExternalInput")
    tmat = nc.dram_tensor("tmat", [C12, OUT], f16, kind="ExternalInput")
    out = nc.dram_tensor("out", [OUT, TOK], f32, kind="ExternalOutput")

    with tile.TileContext(nc) as tc:
        with (
            tc.tile_pool(name="const", bufs=1) as cp,
            tc.tile_pool(name="hp", bufs=hbufs) as hp,
            tc.tile_pool(name="wk", bufs=wkbufs) as wk,
            tc.tile_pool(name="ps", bufs=psbufs, space="PSUM") as ps,
            tc.tile_pool(name="ps2", bufs=2, space="PSUM") as ps2,
        ):
            # all constants ride the scalar HWDGE ring so the sync ring can
            # start streaming H immediately
            wt = cp.tile([P, DCH, C12], f16, name="wt")
            nc.scalar.dma_start(wt[:], wstk.ap().rearrange("(do p) c -> p do c", p=P))
            bv = cp.tile([C12, 1], f32, name="bv")
            nc.scalar.dma_start(bv[:], bvec.ap())
            tt = cp.tile([C12, OUT], f16, name="tt")
            nc.scalar.dma_start(tt[:], tmat.ap())
            msk = cp.tile([C12, TOK], f16, name="msk")
            nc.scalar.dma_start(msk[:], mask.ap())

            hv = h2.ap()

            def body():
                _emit_body(nc, hv, out, hp, wk, ps, ps2, wt, bv, tt, msk,
                           bt, split_dma, dual_ring, mode, g, ostore)

            if repeat == 1:
                body()
            else:
                with tc.For_i(0, repeat, 1):
                    body()

    nc.compile()
    _CACHED_NC[key] = nc
    return nc


def _emit_body(nc, hv, out, hp, wk, ps, ps2, wt, bv, tt, msk,
               bt, split_dma, dual_ring, mode="full", g=G, ostore=1):
    f16 = mybir.dt.float16
    f32 = mybir.dt.float32
    gpb = bt // g   # groups per DMA block
    sgb = [None]    # current batched-output buffer (ostore groups per store)

    def emit_main(ht, gi):
        gs = slice(gi * g, (gi + 1) * g)
        p = ps.tile([C12, g], f32, name="p")
        for do in range(DCH):
            nc.tensor.matmul(
                p[:], wt[:, do, :], ht[:, do, gs],
                start=(do == 0), stop=(do == DCH - 1),
                skip_group_check=True,
            )
        return p

    def emit_select(p, off):
        # m = f16((p + bias) * mask); one pass on VectorE, then a single f16
        # pair-sum matmul with the constant 0/1 matrix lands out[2, G] in
        # PSUM, stored straight to DRAM from there.
        m = wk.tile([C12, g], f16, name="m")
        nc.vector.scalar_tensor_tensor(
            m[:], p[:], bv[:, 0:1], msk[:, off:off + g],
            mybir.AluOpType.add, mybir.AluOpType.mult,
        )
        p2 = ps2.tile([OUT, g], f32, name="p2")
        nc.tensor.matmul(
            p2[:], tt[:], m[:], start=True, stop=True, skip_group_check=True,
        )
        # PSUM -> SBUF on the (otherwise idle) Activation engine; store every
        # `ostore` groups so fewer store DMAs contend with the H-load stream
        j = off // g
        if j % ostore == 0:
            sgb[0] = wk.tile([OUT, ostore * g], f32, name="sg")
        slot = (j % ostore) * g
        nc.scalar.copy(sgb[0][:, slot:slot + g], p2[:])
        if j % ostore == ostore - 1:
            base = (j - (ostore - 1)) * g
            nc.scalar.dma_start(out.ap()[:, base:base + ostore * g], sgb[0][:])

    if mode == "compute":
        ht0 = hp.tile([P, DCH, bt], f16, name="ht", bufs=1)
        nc.sync.dma_start(ht0[:], hv[0])
        pending = None
        for c in range(TOK // bt):
            for gi in range(gpb):
                p = emit_main(ht0, gi)
                if pending is not None:
                    emit_select(*pending)
                pending = (p, c * bt + gi * g)
        emit_select(*pending)
        return

    pending = None
    for c in range(TOK // bt):
        ht = hp.tile([P, DCH, bt], f16, name="ht")
        ring = nc.scalar if (dual_ring and c % 2) else nc.sync
        if split_dma:
            ring.dma_start(ht[:, : DCH // 2], hv[c, :, : DCH // 2])
            ring.dma_start(ht[:, DCH // 2:], hv[c, :, DCH // 2:])
        else:
            ring.dma_start(ht[:], hv[c])
        if mode == "dma":
            # keep a reader so buffers recycle without stalling the queue
            nc.vector.tensor_copy(msk[0:1, 0:8], ht[0:1, 0, 0:8])
            continue
        for gi in range(gpb):
            p = emit_main(ht, gi)
            if pending is not None:
                emit_select(*pending)
            pending = (p, c * bt + gi * g)
    if mode == "dma":
        return
    emit_select(*pending)


def _host_prep(H, bus_type, W, b, bt=BT):
    """Shard + precision-split inputs; returns per-core in_maps."""
    H = np.asarray(H, dtype=np.float32)
    W = np.asarray(W, dtype=np.float32)
    b = np.asarray(b, dtype=np.float32)
    types = np.asarray(bus_type)[0].astype(np.int64)  # decoder choice = row 0

    # Weight stack [D, 12]: cols 2t+o = Whi[t,:,o], cols 6+2t+o = Wlo[t,:,o]
    W6 = np.ascontiguousarray(W.transpose(1, 0, 2).reshape(D, 2 * N_TYPES))
    Whi = W6.astype(np.float16)
    Wlo = (W6 - Whi.astype(np.float32)).astype(np.float16)
    wstk = np.ascontiguousarray(np.concatenate([Whi, Wlo], axis=1))

    # Exact f32 bias, applied per-partition on VectorE before the mask-mul
    bvec = np.zeros((C12, 1), np.float32)
    bvec[0:2 * N_TYPES, 0] = b.reshape(2 * N_TYPES)

    # One-hot routing mask per token (token j = b_local*N + n -> depends on n)
    oh = (types[None, :] == np.arange(N_TYPES)[:, None])      # [3, N]
    m6 = np.repeat(oh, 2, axis=0)                             # [6, N]
    m6t = np.tile(m6, (1, BL)).astype(np.float16)             # [6, TOK]
    mask12 = np.ascontiguousarray(np.concatenate([m6t, m6t], axis=0))

    # Constant pair-sum matrix: out[o] = sum_{c: c%2==o} m[c] (exact in f16)
    tmat = np.zeros((C12, OUT), np.float16)
    tmat[0::2, 0] = 1.0
    tmat[1::2, 1] = 1.0

    def pretile(arr):
        # [D, TOK] -> [NB, P, DCH, bt]: one contiguous run per partition
        return np.ascontiguousarray(
            arr.reshape(DCH, P, TOK // bt, bt)
               .transpose(2, 1, 0, 3)
               .reshape(TOK // bt, P, DCH, bt)
        )

    in_maps = []
    for ci in range(N_CORES):
        Hc = np.ascontiguousarray(H[ci * BL:(ci + 1) * BL].reshape(TOK, D).T)
        im = {
            "h2": pretile(Hc.astype(np.float16)),
            "wstk": wstk,
            "bvec": bvec,
            "mask12": mask12,
            "tmat": tmat,
        }
        in_maps.append(im)
    return in_maps


def _unshard(results):
    outs = []
    for ci in range(N_CORES):
        ot = results[ci]["out"]  # [2, TOK] f32
        outs.append(ot.reshape(OUT, BL, N).transpose(1, 2, 0).reshape(BL, N * OUT))
    return np.ascontiguousarray(np.concatenate(outs, axis=0).astype(np.float32))


def kernel(H, bus_type, W, b):
    nc = _build_nc()
    in_maps = _host_prep(H, bus_type, W, b)
    res = bass_utils.run_bass_kernel_spmd(
        nc, in_maps, core_ids=list(range(N_CORES))
    )
    return _unshard(res.results)


if __name__ == "__main__":
    rng = np.random.default_rng(0)
    H = rng.standard_normal((B, N, D)).astype(np.float32)
    bus_type = rng.integers(0, N_TYPES, size=(B, N)).astype(np.int64)
    W = rng.uniform(-1 / 32, 1 / 32, size=(N_TYPES, D, OUT)).astype(np.float32)
    b = rng.uniform(-1 / 32, 1 / 32, size=(N_TYPES, OUT)).astype(np.float32)
    got = kernel(H, bus_type, W, b)
    types = bus_type[0]
    want = (np.einsum("bnd,ndo->bno", H, W[types]) + b[types][None]).reshape(B, -1)
    err = np.abs(got - want)
    print("max abs err:", err.max(), "absmax-rel:", err.max() / np.abs(want).max())


# revision 17
# speedup vs baseline: 1.0250x; 1.0250x over previous
"""Trainium2 Bass kernel for nn_BusDecoder (moe_routing).

Computes out[b, n*2+o] = sum_d H[b,n,d] * W[t_n, d, o] + b[t_n, o] with
t_n = bus_type[0, n], for B=32, N=4096, D=1024, OUT=2, 3 types.

Strategy (memory-bound regime):
  - Data-parallel over batch B across 8 cores (B_local=4 per core).
  - H streams as f16 (absmax-rel err ~2e-4 vs the 2e-2 gate); W rides as a
    12-column f16 stack [Whi | Wlo] with Wlo the f32-f16 residual, so the
    einsum sees W at effectively full f32 precision for free (the mask stage
    sums both halves).
  - H is pre-tiled on the host so every DMA block is one contiguous run per
    partition (this is the main memory-rate lever).
  - Per 512-token group: 8 accumulating f16 matmuls [K=128, M=12, N=512]
    into PSUM, then one VectorE scalar_tensor_tensor applies bias + one-hot
    routing mask (by bus type) writing f16 directly, and a single f16 matmul
    with a constant 0/1 matrix T[12, 2] pair-sums into out[2, 512] in PSUM;
    the Activation engine copies PSUM->SBUF and stores are batched 8 groups
    per DMA on the scalar HWDGE ring so they rarely contend with H loads on
    the shared SDMA engines. The select for group g is emitted one group
    late so the PE never waits on VectorE.
  - Deep buffering (8x 2MB H-block buffers, 4 PSUM groups) keeps the
    sync-ring DMA queue fed end-to-end; measured per-exec time tracks the
    pure-DMA rate of the H stream (~385-437 GB/s/core per machine load).
"""

import numpy as np

import concourse.bacc as bacc
import concourse.bass_utils as bass_utils
import concourse.mybir as mybir
import concourse.tile as tile

B, N, D, OUT = 32, 4096, 1024, 2
N_TYPES = 3
N_CORES = 8
BL = B // N_CORES          # 4 batch rows per core
TOK = BL * N               # 16384 tokens per core
P = 128
DCH = D // P               # 8 contraction chunks
G = 512                    # tokens per matmul group (one PSUM bank of fp32)
NG = TOK // G              # 32 groups
C12 = 12                   # weight stack width (Whi | Wlo)

BT = 1024                  # tokens per H DMA block (host pretile granularity)

_CACHED_NC = {}


def _build_nc(repeat=1, bt=BT, hbufs=8, split_dma=False, psbufs=4,
              dual_ring=False, mode="full", g=G, wkbufs=2, ostore=8):
    # repeat>1 wraps the body in a device-side For_i loop running the
    # identical workload `repeat` times — used only by test.py to measure
    # per-execution hardware time through the high-latency axon tunnel.
    # mode: "full" | "dma" (loads only) | "compute" (loads once, loops math)
    key = (repeat, bt, hbufs, split_dma, psbufs, dual_ring, mode, g, wkbufs, ostore)
    if key in _CACHED_NC:
        return _CACHED_NC[key]

    f16 = mybir.dt.float16
    f32 = mybir.dt.float32

    nc = bacc.Bacc("TRN2", debug=False)
    # h2 is host-pre-tiled to the exact per-block SBUF layout so each block
    # DMA reads one contiguous run per partition:
    #   h2[c, p, do, t] = (H^T)[do*128+p, c*BT+t]
    h2 = nc.dram_tensor("h2", [TOK // bt, P, DCH, bt], f16,
                        kind="ExternalInput")
    wstk = nc.dram_tensor("wstk", [D, C12], f16, kind="ExternalInput")
    bvec = nc.dram_tensor("bvec", [C12, 1], f32, kind="ExternalInput")
    mask = nc.dram_tensor("mask12", [C12, TOK], f16, kind="ExternalInput")
    tmat = nc.dram_tensor("tmat", [C12, OUT], f16, kind="ExternalInput")
    out = nc.dram_tensor("out", [OUT, TOK], f32, kind="ExternalOutput")

    with tile.TileContext(nc) as tc:
        with (
            tc.tile_pool(name="const", bufs=1) as cp,
            tc.tile_pool(name="hp", bufs=hbufs) as hp,
            tc.tile_pool(name="wk", bufs=wkbufs) as wk,
            tc.tile_pool(name="ps", bufs=psbufs, space="PSUM") as ps,
            tc.tile_pool(name="ps2", bufs=2, space="PSUM") as ps2,
        ):
            # all constants ride the scalar HWDGE ring so the sync ring can
            # start streaming H immediately
            wt = cp.tile([P, DCH, C12], f16, name="wt")
            nc.scalar.dma_start(wt[:], wstk.ap().rearrange("(do p) c -> p do c", p=P))
            bv = cp.tile([C12, 1], f32, name="bv")
            nc.scalar.dma_start(bv[:], bvec.ap())
            tt = cp.tile([C12, OUT], f16, name="tt")
            nc.scalar.dma_start(tt[:], tmat.ap())
            msk = cp.tile([C12, TOK], f16, name="msk")
            nc.scalar.dma_start(msk[:], mask.ap())

            hv = h2.ap()

            def body():
                _emit_body(nc, hv, out, hp, wk, ps, ps2, wt, bv, tt, msk,
                           bt, split_dma, dual_ring, mode, g, ostore)

            if repeat == 1:
                body()
            else:
                with tc.For_i(0, repeat, 1):
                    body()

    nc.compile()
    _CACHED_NC[key] = nc
    return nc


def _emit_body(nc, hv, out, hp, wk, ps, ps2, wt, bv, tt, msk,
               bt, split_dma, dual_ring, mode="full", g=G, ostore=1):
    f16 = mybir.dt.float16
    f32 = mybir.dt.float32
    gpb = bt // g   # groups per DMA block
    sgb = [None]    # current batched-output buffer (ostore groups per store)

    def emit_main(ht, gi):
        gs = slice(gi * g, (gi + 1) * g)
        p = ps.tile([C12, g], f32, name="p")
        for do in range(DCH):
            nc.tensor.matmul(
                p[:], wt[:, do, :], ht[:, do, gs],
                start=(do == 0), stop=(do == DCH - 1),
                skip_group_check=True,
            )
        return p

    def emit_select(p, off):
        # m = f16((p + bias) * mask); one pass on VectorE, then a single f16
        # pair-sum matmul with the constant 0/1 matrix lands out[2, G] in
        # PSUM, stored straight to DRAM from there.
        m = wk.tile([C12, g], f16, name="m")
        nc.vector.scalar_tensor_tensor(
            m[:], p[:], bv[:, 0:1], msk[:, off:off + g],
            mybir.AluOpType.add, mybir.AluOpType.mult,
        )
        p2 = ps2.tile([OUT, g], f32, name="p2")
        nc.tensor.matmul(
            p2[:], tt[:], m[:], start=True, stop=True, skip_group_check=True,
        )
        # PSUM -> SBUF on the (otherwise idle) Activation engine; store every
        # `ostore` groups so fewer store DMAs contend with the H-load stream
        j = off // g
        if j % ostore == 0:
            sgb[0] = wk.tile([OUT, ostore * g], f32, name="sg")
        slot = (j % ostore) * g
        nc.scalar.copy(sgb[0][:, slot:slot + g], p2[:])
        if j % ostore == ostore - 1:
            base = (j - (ostore - 1)) * g
            nc.scalar.dma_start(out.ap()[:, base:base + ostore * g], sgb[0][:])

    if mode == "compute":
        ht0 = hp.tile([P, DCH, bt], f16, name="ht", bufs=1)
        nc.sync.dma_start(ht0[:], hv[0])
        pending = None
        for c in range(TOK // bt):
            for gi in range(gpb):
                p = emit_main(ht0, gi)
                if pending is not None:
                    emit_select(*pending)
                pending = (p, c * bt + gi * g)
        emit_select(*pending)
        return

    pending = None
    for c in range(TOK // bt):
        ht = hp.tile([P, DCH, bt], f16, name="ht")
        ring = nc.scalar if (dual_ring and c % 2) else nc.sync
        if split_dma:
            ring.dma_start(ht[:, : DCH // 2], hv[c, :, : DCH // 2])
            ring.dma_start(ht[:, DCH // 2:], hv[c, :, DCH // 2:])
        else:
            ring.dma_start(ht[:], hv[c])
        if mode == "dma":
            # keep a reader so buffers recycle without stalling the queue
            nc.vector.tensor_copy(msk[0:1, 0:8], ht[0:1, 0, 0:8])
            continue
        for gi in range(gpb):
            p = emit_main(ht, gi)
            if pending is not None:
                emit_select(*pending)
            pending = (p, c * bt + gi * g)
    if mode == "dma":
        return
    emit_select(*pending)


def _host_prep(H, bus_type, W, b, bt=BT):
    """Shard + precision-split inputs; returns per-core in_maps."""
    H = np.asarray(H, dtype=np.float32)
    W = np.asarray(W, dtype=np.float32)
    b = np.asarray(b, dtype=np.float32)
    types = np.asarray(bus_type)[0].astype(np.int64)  # decoder choice = row 0

    # Weight stack [D, 12]: cols 2t+o = Whi[t,:,o], cols 6+2t+o = Wlo[t,:,o]
    W6 = np.ascontiguousarray(W.transpose(1, 0, 2).reshape(D, 2 * N_TYPES))
    Whi = W6.astype(np.float16)
    Wlo = (W6 - Whi.astype(np.float32)).astype(np.float16)
    wstk = np.ascontiguousarray(np.concatenate([Whi, Wlo], axis=1))

    # Exact f32 bias, applied per-partition on VectorE before the mask-mul
    bvec = np.zeros((C12, 1), np.float32)
    bvec[0:2 * N_TYPES, 0] = b.reshape(2 * N_TYPES)

    # One-hot routing mask per token (token j = b_local*N + n -> depends on n)
    oh = (types[None, :] == np.arange(N_TYPES)[:, None])      # [3, N]
    m6 = np.repeat(oh, 2, axis=0)                             # [6, N]
    m6t = np.tile(m6, (1, BL)).astype(np.float16)             # [6, TOK]
    mask12 = np.ascontiguousarray(np.concatenate([m6t, m6t], axis=0))

    # Constant pair-sum matrix: out[o] = sum_{c: c%2==o} m[c] (exact in f16)
    tmat = np.zeros((C12, OUT), np.float16)
    tmat[0::2, 0] = 1.0
    tmat[1::2, 1] = 1.0

    def pretile(arr):
        # [D, TOK] -> [NB, P, DCH, bt]: one contiguous run per partition
        return np.ascontiguousarray(
            arr.reshape(DCH, P, TOK // bt, bt)
               .transpose(2, 1, 0, 3)
               .reshape(TOK // bt, P, DCH, bt)
        )

    in_maps = []
    for ci in range(N_CORES):
        Hc = np.ascontiguousarray(H[ci * BL:(ci + 1) * BL].reshape(TOK, D).T)
        im = {
            "h2": pretile(Hc.astype(np.float16)),
            "wstk": wstk,
            "bvec": bvec,
            "mask12": mask12,
            "tmat": tmat,
        }
        in_maps.append(im)
    return in_maps


def _unshard(results):
    outs = []
    for ci in range(N_CORES):
        ot = results[ci]["out"]  # [2, TOK] f32
        outs.append(ot.reshape(OUT, BL, N).transpose(1, 2, 0).reshape(BL, N * OUT))
    return np.ascontiguousarray(np.concatenate(outs, axis=0).astype(np.float32))


def kernel(H, bus_type, W, b):
    nc = _build_nc()
    in_maps = _host_prep(H, bus_type, W, b)
    res = bass_utils.run_bass_kernel_spmd(
        nc, in_maps, core_ids=list(range(N_CORES))
    )
    return _unshard(res.results)


if __name__ == "__main__":
    rng = np.random.default_rng(0)
    H = rng.standard_normal((B, N, D)).astype(np.float32)
    bus_type = rng.integers(0, N_TYPES, size=(B, N)).astype(np.int64)
    W = rng.uniform(-1 / 32, 1 / 32, size=(N_TYPES, D, OUT)).astype(np.float32)
    b = rng.uniform(-1 / 32, 1 / 32, size=(N_TYPES, OUT)).astype(np.float32)
    got = kernel(H, bus_type, W, b)
    types = bus_type[0]
    want = (np.einsum("bnd,ndo->bno", H, W[types]) + b[types][None]).reshape(B, -1)
    err = np.abs(got - want)
    print("max abs err:", err.max(), "absmax-rel:", err.max() / np.abs(want).max())
